# revision 18
# baseline (speedup 1.0000x reference)
"""Trainium2 Bass kernel for masked multi-head attention (8-core SPMD).

Problem: B=2, S=2048, d_in=hid=512, H=8 heads (dh=64), fp32 in/out.
Reference quirk: the mask uses np.tile(valid_length, H), so scores row
i = b*H + h is masked with valid_length[h % 2] — head PARITY, not batch.

Sharding (8 cores): core c = (batch b = c//4, head-pair p = c%4).
Each core computes heads {2p, 2p+1} of batch b over all 2048 queries,
producing a partial [2048, 512] through its 128 rows of Wo; the host
sums the 4 pair-partials per batch.

Design (v4):
- bf16 on the wire and on-chip (host casts inputs); PSUM stays fp32.
- inputs land part-major as 2 large DMAs per tensor, issued on BOTH
  hardware DGE queues (sync + scalar) so transfers overlap; weights
  are packed into one tensor. v3 serialized 35 issues on one queue and
  the first matmul waited 20us.
- attention loop (query-half, head, key-tile): one kT weight load
  feeds 2 score MMs, one 1024-wide ACTIVATE per key tile, PV
  accumulates into a 2-bank PSUM tile.
- the k/v projections, v-transposes, and Wo(qh0) are INTERLEAVED into
  the attention kt loops (deadline-driven), so the PE stream stays
  dense — v3 ran 67% of the kernel HAM-throttled at 1.2 GHz because
  the ACT-bound attention loop left periodic PE idle gaps.
- masking is baked into v_aug: columns 64:128 are ones (PV emits the
  softmax denominator on partitions 64:128) and masked key rows are
  zeroed via a keep-vector input, so masked keys drop out of both
  numerator and denominator — no exp bias anywhere. exp(junk) is
  finite and multiplied by zero.
- normalize: DVE copy of denominator rows to SBUF, then
  reciprocal_approx_fast SBUF->SBUF at partition base 0 (rafast
  directly on PSUM at base 64 returned garbage on HW), then one
  tensor_mul.
- all PSUM work shares one rotating 2-bank tag (sc) + a 2-bank pv
  tag: 8 banks exactly, both double-buffered.
"""

import math
import os

import ml_dtypes
import numpy as np

from concourse import bacc
import concourse.mybir as mybir
import concourse.tile as tile
from concourse.bass_utils import run_bass_kernel_spmd
from concourse.masks import make_identity

F32 = mybir.dt.float32
BF16 = mybir.dt.bfloat16
EXP = mybir.ActivationFunctionType.Exp

B, S, D, HID, H, DH = 2, 2048, 512, 512, 8, 64


def _build(nkt_e: int, nkt_o: int):
    """One BIR program, same on all 8 cores. nkt_e/nkt_o = number of
    128-key tiles for the even/odd head (from vl[0]/vl[1])."""
    nc = bacc.Bacc("TRN2", target_bir_lowering=False, debug=False,
                   num_devices=8)
    NKT = (nkt_e, nkt_o)
    NKTM = max(NKT)
    KMAX = NKTM * 128
    NCH = (KMAX + 511) // 512          # k/v projection chunks
    KH = min(1024, KMAX)               # first-half split for k/v DMAs
    LONG = 0 if nkt_e >= nkt_o else 1  # head with more key tiles

    qT_d = nc.dram_tensor("qT", [128, 4, S], BF16, kind="ExternalInput").ap()
    kT_d = nc.dram_tensor("kT", [128, 4, KMAX], BF16, kind="ExternalInput").ap()
    vT_d = nc.dram_tensor("vT", [128, 4, KMAX], BF16, kind="ExternalInput").ap()
    wqkv_d = nc.dram_tensor("wqkv", [128, 12, 128], BF16,
                            kind="ExternalInput").ap()
    wo_d = nc.dram_tensor("wo", [128, 512], BF16, kind="ExternalInput").ap()
    keep_d = [nc.dram_tensor(f"keep{l}", [128, 1], F32,
                             kind="ExternalInput").ap() for l in range(2)]
    out_d = nc.dram_tensor("out", [16, 128, 512], BF16,
                           kind="ExternalOutput").ap()

    with tile.TileContext(nc) as tc:
        with (
            tc.tile_pool(name="consts", bufs=1) as consts,
            tc.tile_pool(name="inputs", bufs=1) as inputs,
            tc.tile_pool(name="work", bufs=1) as work,
            tc.tile_pool(name="exps", bufs=4) as exps,
            tc.tile_pool(name="recp", bufs=2) as recp,
            tc.tile_pool(name="sop", bufs=3) as sop,
            tc.tile_pool(name="psc", bufs=2, space="PSUM") as psc,
            tc.tile_pool(name="ppv", bufs=2, space="PSUM") as ppv,
        ):
            ident = consts.tile([128, 128], F32)
            make_identity(nc, ident[:])
            wqkv_s = consts.tile([128, 12, 128], BF16)
            wo_s = consts.tile([128, 512], BF16)
            keep_s = []
            # scalar-queue DMAs: weights first, then kT halves
            nc.scalar.dma_start(wqkv_s[:], wqkv_d[:])
            for l in range(2):
                m = consts.tile([128, 1], F32, tag=f"keep{l}")
                nc.scalar.dma_start(m[:], keep_d[l][:])
                keep_s.append(m)
            nc.scalar.dma_start(wo_s[:], wo_d[:])

            qT_in = inputs.tile([128, 4, S], BF16)
            kT_in = inputs.tile([128, 4, KMAX], BF16)
            vT_in = inputs.tile([128, 4, KMAX], BF16)
            # ordered by first use: qT both halves (kt0 spans all queries),
            # then v first-half pieces; k goes on the scalar queue in
            # 512-key pieces so kproj c0 unblocks earliest
            KH0 = min(512, KMAX)
            nc.sync.dma_start(qT_in[:, :, 0:1024], qT_d[:, :, 0:1024])
            nc.scalar.dma_start(kT_in[:, :, 0:KH0], kT_d[:, :, 0:KH0])
            nc.sync.dma_start(qT_in[:, :, 1024:S], qT_d[:, :, 1024:S])
            if KMAX > KH0:
                nc.scalar.dma_start(kT_in[:, :, KH0:KH], kT_d[:, :, KH0:KH])
            nc.sync.dma_start(vT_in[:, :, 0:KH0], vT_d[:, :, 0:KH0])
            if KMAX > KH0:
                nc.sync.dma_start(vT_in[:, :, KH0:KH], vT_d[:, :, KH0:KH])
            for pos in range(KH, KMAX, 512):
                hi = min(pos + 512, KMAX)
                nc.scalar.dma_start(kT_in[:, :, pos:hi], kT_d[:, :, pos:hi])
                nc.scalar.dma_start(vT_in[:, :, pos:hi], vT_d[:, :, pos:hi])

            qTp = work.tile([128, S], BF16)      # [2*64 head rows, q]
            kTp = work.tile([128, KMAX], BF16)   # rows l*64.., keys
            vTp = work.tile([128, KMAX], F32)
            vaug = work.tile([128, NKTM, 2, 128], BF16)
            outT = work.tile([128, S], BF16)
            nc.vector.memset(vaug[:, :, :, 64:128], 1.0)

            def mix_tile(name):
                return psc.tile([128, 1024], F32, tag="sc", name=name)

            cp_s = nc.scalar.copy               # ACT copy (prologue)
            cp_v = nc.vector.tensor_copy        # DVE copy (interleaved)

            def emit_qproj(c, cp):
                ps = mix_tile("psq")
                for dt in range(4):
                    nc.tensor.matmul(ps[:, 0:512], wqkv_s[:, dt],
                                     qT_in[:, dt, c * 512:(c + 1) * 512],
                                     start=(dt == 0), stop=(dt == 3))
                cp(qTp[:, c * 512:(c + 1) * 512], ps[:, 0:512])

            def emit_kvproj(which, c, cp):
                pos = c * 512
                ncols = min(512, KMAX - pos)
                ps = mix_tile("pskv")
                o = ps[:, 0:ncols]
                for dt in range(4):
                    nc.tensor.matmul(o, wqkv_s[:, 4 * (1 + which) + dt],
                                     (kT_in if which == 0 else vT_in)
                                     [:, dt, pos:pos + ncols],
                                     start=(dt == 0), stop=(dt == 3))
                cp((kTp if which == 0 else vTp)[:, pos:pos + ncols], o)

            def emit_tp(kt):
                ps = mix_tile("ptp")
                nc.tensor.transpose(ps[:, 0:128],
                                    vTp[:, kt * 128:(kt + 1) * 128], ident[:])
                nc.vector.tensor_copy(
                    vaug[:, kt, :, 0:64],
                    ps[:, 0:128].rearrange("p (h d) -> p h d", h=2))

            def emit_keepmul(l):
                nc.gpsimd.tensor_scalar_mul(
                    vaug[:, NKT[l] - 1, l, :], vaug[:, NKT[l] - 1, l, :],
                    keep_s[l][:])

            def emit_wo(qt, cp):
                ps = mix_tile("po")
                nc.tensor.matmul(ps[:, 0:512], outT[:, qt * 128:(qt + 1) * 128],
                                 wo_s[:], start=True, stop=True)
                so = sop.tile([128, 512], BF16, tag="so", name="so")
                cp(so[:], ps[:, 0:512])
                eng = nc.sync if qt % 2 == 0 else nc.scalar
                eng.dma_start(out_d[qt], so[:])

            # ---- prologue, ordered by DMA arrival ----
            NPRO = min(2, NCH)           # chunks covered by the first halves
            for c in range(2):
                emit_qproj(c, cp_s)
            emit_kvproj(0, 0, cp_s)
            for c in range(2, 4):
                emit_qproj(c, cp_s)
            emit_kvproj(1, 0, cp_s)
            for kt in range(min(4, NKTM)):
                emit_tp(kt)
            if NPRO > 1:
                emit_kvproj(0, 1, cp_s)
                emit_kvproj(1, 1, cp_s)
                for kt in range(4, min(8, NKTM)):
                    emit_tp(kt)
            for l in range(2):
                if NKT[l] - 1 < 4 * NPRO:
                    emit_keepmul(l)

            # deadline-tagged pending work, interleaved into qh0's long head
            pend = []
            for c in range(NPRO, NCH):
                pend.append((lambda c=c: emit_kvproj(0, c, cp_v), 4 * c))
                pend.append((lambda c=c: emit_kvproj(1, c, cp_v), 4 * c))
                for kt in range(4 * c, min(4 * c + 4, NKTM)):
                    pend.append((lambda kt=kt: emit_tp(kt), kt))
                    for l in range(2):
                        if NKT[l] - 1 == kt:
                            pend.append((lambda l=l: emit_keepmul(l), kt))
            pend += [(lambda c=c: emit_qproj(c, cp_v), 10 ** 6)
                     for c in range(2, 4)]

            def drain_pend(i):
                # emit everything due before attention kt i+1, plus one
                while pend and pend[0][1] <= i + 1:
                    pend.pop(0)[0]()
                if pend:
                    pend.pop(0)[0]()

            wo_pend = []

            def attention(qh, l, interleave, pending_fin=None):
                nkt = NKT[l]
                pv = ppv.tile([128, 1024], F32, tag="pv", name="pv")

                def emit_pv(kt, es):
                    for j in range(2):
                        nc.tensor.matmul(
                            pv[:, j * 512:(j + 1) * 512], vaug[:, kt, l, :],
                            es[:, j * 512:(j + 1) * 512],
                            start=(kt == 0), stop=(kt == nkt - 1))

                prev = None
                for kt in range(nkt):
                    sc = psc.tile([128, 1024], F32, tag="sc", name="sc")
                    for j in range(2):
                        qc = qh * 2 + j
                        nc.tensor.matmul(
                            sc[:, j * 512:(j + 1) * 512],
                            kTp[l * 64:(l + 1) * 64, kt * 128:(kt + 1) * 128],
                            qTp[l * 64:(l + 1) * 64, qc * 512:(qc + 1) * 512],
                            start=True, stop=True)
                    es = exps.tile([128, 1024], BF16, tag="es", name="es")
                    nc.scalar.activation(es[:], sc[:], EXP, scale=0.125)
                    # PV runs one stage behind exp so the PE never waits
                    # on the current tile's ACT; the PREVIOUS head's final
                    # PV + normalize are deferred to our kt0 so the PE
                    # pipeline never drains at a head seam
                    if prev is not None:
                        emit_pv(*prev)
                    elif pending_fin is not None:
                        pending_fin()
                    prev = (kt, es)
                    if interleave == "pend":
                        drain_pend(kt)
                    elif interleave == "wo" and kt >= 6 and wo_pend:
                        emit_wo(wo_pend.pop(0), cp_v)

                def fin():
                    emit_pv(*prev)
                    dens = recp.tile([64, 1024], F32, tag="dens", name="dens")
                    nc.vector.tensor_copy(dens[:], pv[64:128, :])
                    rec = recp.tile([64, 1024], F32, tag="rec", name="rec")
                    nc.vector.reciprocal_approx_fast(rec[:], dens[:])
                    nc.vector.tensor_mul(
                        outT[l * 64:(l + 1) * 64, qh * 1024:(qh + 1) * 1024],
                        pv[0:64, :], rec[:])
                return fin

            def attention2(l, interleave, pending_fins):
                # one pass over key tiles; both query halves per tile.
                # 2x ACT work per tile makes the loop comfortably slower
                # than the k/v DMA stream, so no mid-loop data stalls —
                # the two-pass (per-half) order stalled ~10us on late
                # k/v halves and re-throttled the PE clock.
                nkt = NKT[l]
                pvs = [ppv.tile([128, 1024], F32, tag="pv", name=f"pv{qh}")
                       for qh in range(2)]
                prevs = [None, None]

                def emit_pv(qh, kt, es):
                    for j in range(2):
                        nc.tensor.matmul(
                            pvs[qh][:, j * 512:(j + 1) * 512],
                            vaug[:, kt, l, :],
                            es[:, j * 512:(j + 1) * 512],
                            start=(kt == 0), stop=(kt == nkt - 1))

                for kt in range(nkt):
                    for qh in range(2):
                        sc = psc.tile([128, 1024], F32, tag="sc", name="sc")
                        for j in range(2):
                            qc = qh * 2 + j
                            nc.tensor.matmul(
                                sc[:, j * 512:(j + 1) * 512],
                                kTp[l * 64:(l + 1) * 64,
                                    kt * 128:(kt + 1) * 128],
                                qTp[l * 64:(l + 1) * 64,
                                    qc * 512:(qc + 1) * 512],
                                start=True, stop=True)
                        es = exps.tile([128, 1024], BF16, tag="es", name="es")
                        nc.scalar.activation(es[:], sc[:], EXP, scale=0.125)
                        if prevs[qh] is not None:
                            emit_pv(qh, *prevs[qh])
                        elif pending_fins:
                            pending_fins.pop(0)()
                        prevs[qh] = (kt, es)
                    if interleave == "pend":
                        drain_pend(kt)

                def mk_fin(qh):
                    def fin(half_hook=None):
                        emit_pv(qh, *prevs[qh])
                        # per-512-query-half normalize so the caller can
                        # drain Wo for half j while half j+1 normalizes
                        for j in range(2):
                            dens = recp.tile([64, 512], F32, tag="dens",
                                             name="dens")
                            nc.vector.tensor_copy(
                                dens[:],
                                pvs[qh][64:128, j * 512:(j + 1) * 512])
                            rec = recp.tile([64, 512], F32, tag="rec",
                                            name="rec")
                            nc.vector.reciprocal_approx_fast(rec[:], dens[:])
                            nc.vector.tensor_mul(
                                outT[l * 64:(l + 1) * 64,
                                     (qh * 2 + j) * 512:
                                     (qh * 2 + j + 1) * 512],
                                pvs[qh][0:64, j * 512:(j + 1) * 512], rec[:])
                            if half_hook is not None:
                                half_hook(j)
                    return fin
                return [mk_fin(0), mk_fin(1)]

            # ---- attention + Wo ----
            # short head first; its finales (PV flush + normalize) are
            # deferred into the long head's kt0 slots so the PE pipeline
            # never drains at the head seam.
            if NKT[1 - LONG] <= 4 * NPRO:
                fins = attention2(1 - LONG, None, [])
            else:
                # exotic case: short head longer than the prologue's
                # transpose coverage — project everything first
                while pend:
                    pend.pop(0)[0]()
                fins = attention2(1 - LONG, None, [])
            fins_l = attention2(LONG, "pend", list(fins))
            while pend:
                pend.pop(0)[0]()
            fins_l[0](lambda j: [emit_wo(qt, cp_s)
                                 for qt in range(4 * j, 4 * j + 4)])
            fins_l[1](lambda j: [emit_wo(qt, cp_s)
                                 for qt in range(8 + 4 * j, 12 + 4 * j)])
    nc.compile()
    return nc


_CACHE: dict = {}


def kernel(query, key, value, Wq, Wk, Wv, Wo, valid_length):
    query = np.asarray(query, np.float32)
    key = np.asarray(key, np.float32)
    value = np.asarray(value, np.float32)
    Wq = np.asarray(Wq, np.float32); Wk = np.asarray(Wk, np.float32)
    Wv = np.asarray(Wv, np.float32); Wo = np.asarray(Wo, np.float32)
    vl = np.asarray(valid_length).astype(np.int64)
    # head h is masked with vl[h % 2] (reference's np.tile quirk)
    nkt = [max(1, int(math.ceil(int(vl[l]) / 128))) for l in range(2)]

    key_ = (nkt[0], nkt[1])
    if key_ not in _CACHE:
        _CACHE[key_] = _build(*key_)
    nc = _CACHE[key_]
    KMAX = max(nkt) * 128

    bf = lambda a: np.ascontiguousarray(a.astype(ml_dtypes.bfloat16))
    pm = lambda a, n: np.ascontiguousarray(         # [n*128, N] -> [128, n, N]
        a.reshape(n, 128, -1).transpose(1, 0, 2))
    keeps = []
    for l in range(2):
        base = (nkt[l] - 1) * 128
        m = (base + np.arange(128) < int(vl[l])).astype(np.float32)
        keeps.append(np.ascontiguousarray(m.reshape(128, 1)))

    in_maps = []
    for c in range(8):
        b, p = c // 4, c % 4
        wqkv = np.concatenate(
            [pm(bf(W[:, p * 128:(p + 1) * 128]), 4) for W in (Wq, Wk, Wv)],
            axis=1)
        im = {
            "qT": pm(bf(query[b].T), 4),
            "kT": pm(bf(key[b, :KMAX].T), 4),
            "vT": pm(bf(value[b, :KMAX].T), 4),
            "wqkv": np.ascontiguousarray(wqkv),
            "wo": bf(Wo[p * 128:(p + 1) * 128]),
            "keep0": keeps[0], "keep1": keeps[1],
        }
        in_maps.append(im)

    trace = os.environ.get("BASS_KTRACE", "0") == "1"
    kw = dict(trace=True, trace_cores=list(range(8))) if trace else {}
    res = run_bass_kernel_spmd(nc, in_maps, core_ids=list(range(8)), **kw)
    kernel.last_results = res
    out = np.zeros((B, S, HID), np.float32)
    for c in range(8):
        b = c // 4
        r = np.asarray(res.results[c]["out"], dtype=np.float32)
        out[b] += r.reshape(S, HID)
    return out


# revision 19
# speedup vs baseline: 1.0533x; 1.0533x over previous
"""Trainium2 Bass kernel for masked multi-head attention (8-core SPMD).

Problem: B=2, S=2048, d_in=hid=512, H=8 heads (dh=64), fp32 in/out.
Reference quirk: the mask uses np.tile(valid_length, H), so scores row
i = b*H + h is masked with valid_length[h % 2] — head PARITY, not batch.

Sharding (8 cores): core c = (batch b = c//4, head-pair p = c%4).
Each core computes heads {2p, 2p+1} of batch b over all 2048 queries,
producing a partial [2048, 512] through its 128 rows of Wo; the host
sums the 4 pair-partials per batch.

Design (v4):
- bf16 on the wire and on-chip (host casts inputs); PSUM stays fp32.
- inputs land part-major as 2 large DMAs per tensor, issued on BOTH
  hardware DGE queues (sync + scalar) so transfers overlap; weights
  are packed into one tensor. v3 serialized 35 issues on one queue and
  the first matmul waited 20us.
- attention loop (query-half, head, key-tile): one kT weight load
  feeds 2 score MMs, one 1024-wide ACTIVATE per key tile, PV
  accumulates into a 2-bank PSUM tile.
- the k/v projections, v-transposes, and Wo(qh0) are INTERLEAVED into
  the attention kt loops (deadline-driven), so the PE stream stays
  dense — v3 ran 67% of the kernel HAM-throttled at 1.2 GHz because
  the ACT-bound attention loop left periodic PE idle gaps.
- masking is baked into v_aug: columns 64:128 are ones (PV emits the
  softmax denominator on partitions 64:128) and masked key rows are
  zeroed via a keep-vector input, so masked keys drop out of both
  numerator and denominator — no exp bias anywhere. exp(junk) is
  finite and multiplied by zero.
- normalize: DVE copy of denominator rows to SBUF, then
  reciprocal_approx_fast SBUF->SBUF at partition base 0 (rafast
  directly on PSUM at base 64 returned garbage on HW), then one
  tensor_mul.
- all PSUM work shares one rotating 2-bank tag (sc) + a 2-bank pv
  tag: 8 banks exactly, both double-buffered.
"""

import math
import os

import ml_dtypes
import numpy as np

from concourse import bacc
import concourse.mybir as mybir
import concourse.tile as tile
from concourse.bass_utils import run_bass_kernel_spmd
from concourse.masks import make_identity

F32 = mybir.dt.float32
BF16 = mybir.dt.bfloat16
EXP = mybir.ActivationFunctionType.Exp

B, S, D, HID, H, DH = 2, 2048, 512, 512, 8, 64


def _build(nkt_e: int, nkt_o: int):
    """One BIR program, same on all 8 cores. nkt_e/nkt_o = number of
    128-key tiles for the even/odd head (from vl[0]/vl[1])."""
    nc = bacc.Bacc("TRN2", target_bir_lowering=False, debug=False,
                   num_devices=8)
    NKT = (nkt_e, nkt_o)
    NKTM = max(NKT)
    KMAX = NKTM * 128
    NCH = (KMAX + 511) // 512          # k/v projection chunks
    KH = min(1024, KMAX)               # first-half split for k/v DMAs
    LONG = 0 if nkt_e >= nkt_o else 1  # head with more key tiles

    qT_d = nc.dram_tensor("qT", [128, 4, S], BF16, kind="ExternalInput").ap()
    kT_d = nc.dram_tensor("kT", [128, 4, KMAX], BF16, kind="ExternalInput").ap()
    vT_d = nc.dram_tensor("vT", [128, 4, KMAX], BF16, kind="ExternalInput").ap()
    wqkv_d = nc.dram_tensor("wqkv", [128, 12, 128], BF16,
                            kind="ExternalInput").ap()
    wo_d = nc.dram_tensor("wo", [128, 512], BF16, kind="ExternalInput").ap()
    keep_d = [nc.dram_tensor(f"keep{l}", [128, 1], F32,
                             kind="ExternalInput").ap() for l in range(2)]
    out_d = nc.dram_tensor("out", [16, 128, 512], BF16,
                           kind="ExternalOutput").ap()

    with tile.TileContext(nc) as tc:
        with (
            tc.tile_pool(name="consts", bufs=1) as consts,
            tc.tile_pool(name="inputs", bufs=1) as inputs,
            tc.tile_pool(name="work", bufs=1) as work,
            tc.tile_pool(name="exps", bufs=4) as exps,
            tc.tile_pool(name="recp", bufs=2) as recp,
            tc.tile_pool(name="sop", bufs=3) as sop,
            tc.tile_pool(name="psc", bufs=2, space="PSUM") as psc,
            tc.tile_pool(name="ppv", bufs=2, space="PSUM") as ppv,
        ):
            ident = consts.tile([128, 128], F32)
            make_identity(nc, ident[:])
            wqkv_s = consts.tile([128, 12, 128], BF16)
            wo_s = consts.tile([128, 512], BF16)
            keep_s = []
            # scalar-queue DMAs: weights first, then kT halves
            nc.scalar.dma_start(wqkv_s[:], wqkv_d[:])
            for l in range(2):
                m = consts.tile([128, 1], F32, tag=f"keep{l}")
                nc.scalar.dma_start(m[:], keep_d[l][:])
                keep_s.append(m)
            nc.scalar.dma_start(wo_s[:], wo_d[:])

            qT_in = inputs.tile([128, 4, S], BF16)
            kT_in = inputs.tile([128, 4, KMAX], BF16)
            vT_in = inputs.tile([128, 4, KMAX], BF16)
            nc.sync.dma_start(qT_in[:, :, 0:1024], qT_d[:, :, 0:1024])
            nc.scalar.dma_start(kT_in[:, :, 0:KH], kT_d[:, :, 0:KH])
            nc.sync.dma_start(vT_in[:, :, 0:KH], vT_d[:, :, 0:KH])
            nc.sync.dma_start(qT_in[:, :, 1024:S], qT_d[:, :, 1024:S])
            for pos in range(KH, KMAX, 512):
                hi = min(pos + 512, KMAX)
                nc.scalar.dma_start(kT_in[:, :, pos:hi], kT_d[:, :, pos:hi])
                nc.scalar.dma_start(vT_in[:, :, pos:hi], vT_d[:, :, pos:hi])

            qTp = work.tile([128, S], BF16)      # [2*64 head rows, q]
            kTp = work.tile([128, KMAX], BF16)   # rows l*64.., keys
            vTp = work.tile([128, KMAX], F32)
            vaug = work.tile([128, NKTM, 2, 128], BF16)
            outT = work.tile([128, S], BF16)
            nc.vector.memset(vaug[:, :, :, 64:128], 1.0)

            def mix_tile(name):
                return psc.tile([128, 1024], F32, tag="sc", name=name)

            cp_s = nc.scalar.copy               # ACT copy (prologue)
            cp_v = nc.vector.tensor_copy        # DVE copy (interleaved)

            def emit_qproj(c, cp):
                ps = mix_tile("psq")
                for dt in range(4):
                    nc.tensor.matmul(ps[:, 0:512], wqkv_s[:, dt],
                                     qT_in[:, dt, c * 512:(c + 1) * 512],
                                     start=(dt == 0), stop=(dt == 3))
                cp(qTp[:, c * 512:(c + 1) * 512], ps[:, 0:512])

            def emit_kvproj(which, c, cp):
                pos = c * 512
                ncols = min(512, KMAX - pos)
                ps = mix_tile("pskv")
                o = ps[:, 0:ncols]
                for dt in range(4):
                    nc.tensor.matmul(o, wqkv_s[:, 4 * (1 + which) + dt],
                                     (kT_in if which == 0 else vT_in)
                                     [:, dt, pos:pos + ncols],
                                     start=(dt == 0), stop=(dt == 3))
                cp((kTp if which == 0 else vTp)[:, pos:pos + ncols], o)

            def emit_tp(kt):
                ps = mix_tile("ptp")
                nc.tensor.transpose(ps[:, 0:128],
                                    vTp[:, kt * 128:(kt + 1) * 128], ident[:])
                nc.vector.tensor_copy(
                    vaug[:, kt, :, 0:64],
                    ps[:, 0:128].rearrange("p (h d) -> p h d", h=2))

            def emit_keepmul(l):
                nc.gpsimd.tensor_scalar_mul(
                    vaug[:, NKT[l] - 1, l, :], vaug[:, NKT[l] - 1, l, :],
                    keep_s[l][:])

            def emit_wo(qt, cp):
                ps = mix_tile("po")
                nc.tensor.matmul(ps[:, 0:512], outT[:, qt * 128:(qt + 1) * 128],
                                 wo_s[:], start=True, stop=True)
                so = sop.tile([128, 512], BF16, tag="so", name="so")
                cp(so[:], ps[:, 0:512])
                nc.sync.dma_start(out_d[qt], so[:])

            # ---- prologue: q proj (all 4 chunks), first k/v chunks,
            # ---- first transposes ----
            NPRO = min(2, NCH)           # chunks covered by the first halves
            for c in range(2):
                emit_qproj(c, cp_s)
            for c in range(NPRO):
                emit_kvproj(0, c, cp_s)
            for c in range(NPRO):
                emit_kvproj(1, c, cp_s)
            for c in range(2, 4):
                emit_qproj(c, cp_s)
            for kt in range(min(4 * NPRO, NKTM)):
                emit_tp(kt)
            for l in range(2):
                if NKT[l] - 1 < 4 * NPRO:
                    emit_keepmul(l)

            # deadline-tagged pending work, interleaved into qh0's long head
            pend = []
            for c in range(NPRO, NCH):
                pend.append((lambda c=c: emit_kvproj(0, c, cp_v), 4 * c))
                pend.append((lambda c=c: emit_kvproj(1, c, cp_v), 4 * c))
                for kt in range(4 * c, min(4 * c + 4, NKTM)):
                    pend.append((lambda kt=kt: emit_tp(kt), kt))
                    for l in range(2):
                        if NKT[l] - 1 == kt:
                            pend.append((lambda l=l: emit_keepmul(l), kt))
            pend += [(lambda c=c: emit_qproj(c, cp_v), 10 ** 6)
                     for c in range(2, 4)]

            def drain_pend(i):
                # emit everything due before attention kt i+1, plus one
                while pend and pend[0][1] <= i + 1:
                    pend.pop(0)[0]()
                if pend:
                    pend.pop(0)[0]()

            wo_pend = []

            def attention(qh, l, interleave, pending_fin=None):
                nkt = NKT[l]
                pv = ppv.tile([128, 1024], F32, tag="pv", name="pv")

                def emit_pv(kt, es):
                    for j in range(2):
                        nc.tensor.matmul(
                            pv[:, j * 512:(j + 1) * 512], vaug[:, kt, l, :],
                            es[:, j * 512:(j + 1) * 512],
                            start=(kt == 0), stop=(kt == nkt - 1))

                prev = None
                for kt in range(nkt):
                    sc = psc.tile([128, 1024], F32, tag="sc", name="sc")
                    for j in range(2):
                        qc = qh * 2 + j
                        nc.tensor.matmul(
                            sc[:, j * 512:(j + 1) * 512],
                            kTp[l * 64:(l + 1) * 64, kt * 128:(kt + 1) * 128],
                            qTp[l * 64:(l + 1) * 64, qc * 512:(qc + 1) * 512],
                            start=True, stop=True)
                    es = exps.tile([128, 1024], BF16, tag="es", name="es")
                    nc.scalar.activation(es[:], sc[:], EXP, scale=0.125)
                    # PV runs one stage behind exp so the PE never waits
                    # on the current tile's ACT; the PREVIOUS head's final
                    # PV + normalize are deferred to our kt0 so the PE
                    # pipeline never drains at a head seam
                    if prev is not None:
                        emit_pv(*prev)
                    elif pending_fin is not None:
                        pending_fin()
                    prev = (kt, es)
                    if interleave == "pend":
                        drain_pend(kt)
                    elif interleave == "wo" and kt >= 6 and wo_pend:
                        emit_wo(wo_pend.pop(0), cp_v)

                def fin():
                    emit_pv(*prev)
                    dens = recp.tile([64, 1024], F32, tag="dens", name="dens")
                    nc.vector.tensor_copy(dens[:], pv[64:128, :])
                    rec = recp.tile([64, 1024], F32, tag="rec", name="rec")
                    nc.vector.reciprocal_approx_fast(rec[:], dens[:])
                    nc.vector.tensor_mul(
                        outT[l * 64:(l + 1) * 64, qh * 1024:(qh + 1) * 1024],
                        pv[0:64, :], rec[:])
                return fin

            def attention2(l, interleave, pending_fins):
                # one pass over key tiles; both query halves per tile.
                # 2x ACT work per tile makes the loop comfortably slower
                # than the k/v DMA stream, so no mid-loop data stalls —
                # the two-pass (per-half) order stalled ~10us on late
                # k/v halves and re-throttled the PE clock.
                nkt = NKT[l]
                pvs = [ppv.tile([128, 1024], F32, tag="pv", name=f"pv{qh}")
                       for qh in range(2)]
                prevs = [None, None]

                def emit_pv(qh, kt, es):
                    for j in range(2):
                        nc.tensor.matmul(
                            pvs[qh][:, j * 512:(j + 1) * 512],
                            vaug[:, kt, l, :],
                            es[:, j * 512:(j + 1) * 512],
                            start=(kt == 0), stop=(kt == nkt - 1))

                for kt in range(nkt):
                    for qh in range(2):
                        sc = psc.tile([128, 1024], F32, tag="sc", name="sc")
                        for j in range(2):
                            qc = qh * 2 + j
                            nc.tensor.matmul(
                                sc[:, j * 512:(j + 1) * 512],
                                kTp[l * 64:(l + 1) * 64,
                                    kt * 128:(kt + 1) * 128],
                                qTp[l * 64:(l + 1) * 64,
                                    qc * 512:(qc + 1) * 512],
                                start=True, stop=True)
                        es = exps.tile([128, 1024], BF16, tag="es", name="es")
                        nc.scalar.activation(es[:], sc[:], EXP, scale=0.125)
                        if prevs[qh] is not None:
                            emit_pv(qh, *prevs[qh])
                        elif pending_fins:
                            pending_fins.pop(0)()
                        prevs[qh] = (kt, es)
                    if interleave == "pend":
                        drain_pend(kt)

                def mk_fin(qh):
                    def fin(half_hook=None):
                        emit_pv(qh, *prevs[qh])
                        # per-512-query-half normalize so the caller can
                        # drain Wo for half j while half j+1 normalizes
                        for j in range(2):
                            dens = recp.tile([64, 512], F32, tag="dens",
                                             name="dens")
                            nc.vector.tensor_copy(
                                dens[:],
                                pvs[qh][64:128, j * 512:(j + 1) * 512])
                            rec = recp.tile([64, 512], F32, tag="rec",
                                            name="rec")
                            nc.vector.reciprocal_approx_fast(rec[:], dens[:])
                            nc.vector.tensor_mul(
                                outT[l * 64:(l + 1) * 64,
                                     (qh * 2 + j) * 512:
                                     (qh * 2 + j + 1) * 512],
                                pvs[qh][0:64, j * 512:(j + 1) * 512], rec[:])
                            if half_hook is not None:
                                half_hook(j)
                    return fin
                return [mk_fin(0), mk_fin(1)]

            # ---- attention + Wo ----
            # short head first; its finales (PV flush + normalize) are
            # deferred into the long head's kt0 slots so the PE pipeline
            # never drains at the head seam.
            if NKT[1 - LONG] <= 4 * NPRO:
                fins = attention2(1 - LONG, None, [])
            else:
                # exotic case: short head longer than the prologue's
                # transpose coverage — project everything first
                while pend:
                    pend.pop(0)[0]()
                fins = attention2(1 - LONG, None, [])
            fins_l = attention2(LONG, "pend", list(fins))
            while pend:
                pend.pop(0)[0]()
            fins_l[0](lambda j: [emit_wo(qt, cp_s)
                                 for qt in range(4 * j, 4 * j + 4)])
            fins_l[1](lambda j: [emit_wo(qt, cp_s)
                                 for qt in range(8 + 4 * j, 12 + 4 * j)])
    nc.compile()
    return nc


_CACHE: dict = {}


def kernel(query, key, value, Wq, Wk, Wv, Wo, valid_length):
    query = np.asarray(query, np.float32)
    key = np.asarray(key, np.float32)
    value = np.asarray(value, np.float32)
    Wq = np.asarray(Wq, np.float32); Wk = np.asarray(Wk, np.float32)
    Wv = np.asarray(Wv, np.float32); Wo = np.asarray(Wo, np.float32)
    vl = np.asarray(valid_length).astype(np.int64)
    # head h is masked with vl[h % 2] (reference's np.tile quirk)
    nkt = [max(1, int(math.ceil(int(vl[l]) / 128))) for l in range(2)]

    key_ = (nkt[0], nkt[1])
    if key_ not in _CACHE:
        _CACHE[key_] = _build(*key_)
    nc = _CACHE[key_]
    KMAX = max(nkt) * 128

    bf = lambda a: np.ascontiguousarray(a.astype(ml_dtypes.bfloat16))
    pm = lambda a, n: np.ascontiguousarray(         # [n*128, N] -> [128, n, N]
        a.reshape(n, 128, -1).transpose(1, 0, 2))
    keeps = []
    for l in range(2):
        base = (nkt[l] - 1) * 128
        m = (base + np.arange(128) < int(vl[l])).astype(np.float32)
        keeps.append(np.ascontiguousarray(m.reshape(128, 1)))

    in_maps = []
    for c in range(8):
        b, p = c // 4, c % 4
        wqkv = np.concatenate(
            [pm(bf(W[:, p * 128:(p + 1) * 128]), 4) for W in (Wq, Wk, Wv)],
            axis=1)
        im = {
            "qT": pm(bf(query[b].T), 4),
            "kT": pm(bf(key[b, :KMAX].T), 4),
            "vT": pm(bf(value[b, :KMAX].T), 4),
            "wqkv": np.ascontiguousarray(wqkv),
            "wo": bf(Wo[p * 128:(p + 1) * 128]),
            "keep0": keeps[0], "keep1": keeps[1],
        }
        in_maps.append(im)

    trace = os.environ.get("BASS_KTRACE", "0") == "1"
    kw = dict(trace=True, trace_cores=list(range(8))) if trace else {}
    res = run_bass_kernel_spmd(nc, in_maps, core_ids=list(range(8)), **kw)
    kernel.last_results = res
    out = np.zeros((B, S, HID), np.float32)
    for c in range(8):
        b = c // 4
        r = np.asarray(res.results[c]["out"], dtype=np.float32)
        out[b] += r.reshape(S, HID)
    return out
